# revision 20
# baseline (speedup 1.0000x reference)
"""Sliding-window multi-head attention (B=2, S=2048, D=E=768, H=12, window/2=128)
as a Bass/Tile kernel on 8 Trainium2 NeuronCores.

Sharding: data-parallel over batch (2) x tensor-parallel over heads (4 groups
of 3 heads).  Core c handles batch c//4, heads [3*(c%4) .. 3*(c%4)+2].
Each core computes its heads' QKV projection, banded attention, and a partial
output projection (contraction over its 192 features of E); the host sums the
4 partials per batch and adds bo.

Dtypes: fp16 for all matmul operands (fp32 moving operands stream at half
rate; fp16 stationaries get fast-weight-load); bf16 for the attention-weights
x V matmul.  PSUM accumulation fp32.

Device dataflow (per core):
  phase 1: q/k^T feature-major [384, S] via W-stationary matmuls, M-tiles
           [q0|q1][k0|k1][q2|k2]; q2' (partition-base-64 copy of q2 so the
           h2 scores have an aligned moving operand) via SBUF->SBUF DMA.
           V is computed directly token-major: per 128-token block,
           x^T-slice-stationary matmuls x Wv^T -> psum[t,192] -> bf16 V_aug
           (ones columns appended for the softmax denominator).
  phase 2: per key-block kb, per head: scores^T[k,q] = K_kb @ Q^T(window)
           (1/sqrt(hd) folded into Wq on host); exp on ScalarE (padding mask
           as per-partition bias) -> bf16; band mask applied post-exp by DVE
           multiplies of the two triangular edge tiles (0/1 bf16 masks);
           AV with expS^T stationary into a 3-head-shared psum bank ->
           out[q, 3*(64+2)] where col 64 of each slot = denominator;
           normalize via one batched reciprocal + 3 tensor_scalar_muls;
           PE-transpose back to feature-major vals^T (fp16).
  phase 3: partial o^T[e,t], Wo^T-stationary fp16, 2 contract blocks (128+64),
           grouped so the 64-contract matmuls don't stall on psum drain.
"""
import sys

if "/opt/trn_rl_repo" not in sys.path:
    sys.path.insert(0, "/opt/trn_rl_repo")

import numpy as np

B = 2
S = 2048
D = 768
E = 768
H = 12
HD = 64
HALF_WIN = 128  # WINDOW_SIZE // 2
N_CORES = 8
HPC = 3  # heads per core
NEG = -1e30
VS = 66  # v_aug slot width per head: 64 v dims + 2 ones cols

N_TBLK = S // 512      # 4
N_DBLK = D // 128      # 6
N_KB = S // 128        # 16
N_EBLK = E // 128      # 6
N_MT = 3               # q/k M-tiles

_compiled = None


def _build():
    import os
    PHASES = int(os.environ.get("K_PHASES", "3"))
    KIO = int(os.environ.get("K_IO", "3"))
    import concourse.bass as bass
    import concourse.bacc as bacc
    import concourse.mybir as mybir
    import concourse.tile as tile
    from contextlib import ExitStack

    F32 = mybir.dt.float32
    BF16 = mybir.dt.bfloat16
    FP16 = mybir.dt.float16
    AF = mybir.ActivationFunctionType

    nc = bacc.Bacc(None, target_bir_lowering=False)

    xT = nc.dram_tensor("xT", [D, S], FP16, kind="ExternalInput")
    # wqkv f-layout: [q0|q1](128) [k0|k1](128) [q2|k2](128)
    wqkv = nc.dram_tensor("wqkv", [128, N_DBLK, 128 * N_MT], FP16,
                          kind="ExternalInput")
    wv = nc.dram_tensor("wv", [128, N_DBLK, HPC * HD], FP16,
                        kind="ExternalInput")
    wo01 = nc.dram_tensor("wo01", [128, E], FP16, kind="ExternalInput")
    wo2 = nc.dram_tensor("wo2", [64, E], FP16, kind="ExternalInput")
    pmask = nc.dram_tensor("pmask", [128, N_KB], F32, kind="ExternalInput")
    mlo_in = nc.dram_tensor("mlo", [128, 128], BF16, kind="ExternalInput")
    mhi_in = nc.dram_tensor("mhi", [128, 128], BF16, kind="ExternalInput")
    oT = nc.dram_tensor("oT", [E, S], FP16, kind="ExternalOutput")

    with tile.TileContext(nc) as tc, ExitStack() as ctx:
        singles = ctx.enter_context(tc.tile_pool(name="singles", bufs=1))
        epool = ctx.enter_context(tc.tile_pool(name="epool", bufs=6))
        vtpool = ctx.enter_context(tc.tile_pool(name="vtpool", bufs=4))
        rpool = ctx.enter_context(tc.tile_pool(name="rpool", bufs=4))
        ospool = ctx.enter_context(tc.tile_pool(name="ospool", bufs=6))
        psum = ctx.enter_context(tc.tile_pool(name="psum", bufs=1, space="PSUM"))

        # --- resident tiles ---
        wqkv_sb = singles.tile([128, N_DBLK, 128 * N_MT], FP16)
        wv_sb = singles.tile([128, N_DBLK, HPC * HD], FP16)
        wo01_sb = singles.tile([128, E], FP16)
        wo2_sb = singles.tile([64, E], FP16)
        pm_sb = singles.tile([128, N_KB], F32)
        mlo = singles.tile([128, 128], BF16)
        mhi = singles.tile([128, 128], BF16)
        qT01 = singles.tile([128, S], FP16)    # rows 0:64 q0, 64:128 q1
        kT01 = singles.tile([128, S], FP16)    # rows 0:64 k0, 64:128 k1
        qk2T = singles.tile([128, S], FP16)    # rows 0:64 q2, 64:128 k2
        q2p = singles.tile([128, S], FP16)     # rows 64:128 = q2 (DMA copy)
        v_aug = singles.tile([128, N_KB, HPC * VS], BF16)
        valsT01 = singles.tile([128, S], FP16)
        valsT2x = singles.tile([128, S], FP16)  # rows 0:64 = h2 vals^T
        xts = [singles.tile([128, N_DBLK, 512], FP16, name=f"xt{tb}",
                            tag=f"xt{tb}") for tb in range(N_TBLK)]

        mt_dest = [qT01, kT01, qk2T]

        # Interleave wqkv-db and x-tb0 loads across both HWDGE queues so the
        # first M-tile matmuls can start as soon as (db0 weights, db0 x) land.
        # Small attention-phase constants (pm/identh/masks) are issued before
        # the bulk x/wo traffic so the first transposes/exp don't stall.
        # Queue assignment: scalar/ACT gets only tiny constants plus x-tb0 (it
        # must not stall on DMA-ring backpressure — that would block its psum
        # copies); sync gets the critical wqkv/wv; gpsimd (software DGE, idle
        # at kernel start) absorbs all late bulk traffic.
        nc.scalar.dma_start(pm_sb, pmask[:, :])
        nc.sync.dma_start(mlo, mlo_in[:, :])
        nc.sync.dma_start(mhi, mhi_in[:, :])
        hwq = [nc.sync, nc.scalar]
        for db in range(N_DBLK):
            hwq[db % 2].dma_start(wqkv_sb[:, db, :], wqkv[:, db, :])
            if KIO & 1:
                hwq[(db + 1) % 2].dma_start(
                    xts[0][:, db, :], xT[db * 128:(db + 1) * 128, 0:512])
        for db in range(N_DBLK):
            (nc.sync if db % 2 else nc.gpsimd).dma_start(
                wv_sb[:, db, :], wv[:, db, :])
        for tb in range(1, N_TBLK if KIO & 1 else 1):
            for db in range(N_DBLK):
                (nc.gpsimd if db % 2 else nc.sync).dma_start(
                    xts[tb][:, db, :],
                    xT[db * 128:(db + 1) * 128, tb * 512:(tb + 1) * 512])
            if tb == 1:
                nc.gpsimd.dma_start(wo01_sb, wo01[:, :])
                nc.gpsimd.dma_start(wo2_sb, wo2[:, :])
        ones_f32 = singles.tile([128, 1], F32)
        nc.vector.memset(ones_f32, 1.0)
        ones_cols = v_aug.rearrange("p t (h c) -> p t h c", c=VS)[:, :, :, 64:VS]
        ones_src = bass.AP(ones_f32.tensor, ones_f32.offset,
                           [ones_f32.ap[0], [0, N_KB], [0, HPC], [0, VS - 64]])
        nc.scalar.copy(ones_cols, ones_src)

        # ---------------- emission helpers ----------------
        va = v_aug.rearrange("p t (h c) -> p t h c", c=VS)

        def gen_mt(tb, mt):
            cs = slice(tb * 512, (tb + 1) * 512)
            xt = xts[tb]
            ps1 = psum.tile([128, 512], F32, name="ps1", tag=f"mm{mt % 3}")
            for db in range(N_DBLK):
                nc.tensor.matmul(
                    ps1, lhsT=wqkv_sb[:, db, mt * 128:(mt + 1) * 128],
                    rhs=xt[:, db, :],
                    start=(db == 0), stop=(db == N_DBLK - 1))
            if mt % 2 == 0:
                nc.scalar.copy(mt_dest[mt][:, cs], ps1)
            else:
                nc.vector.tensor_copy(mt_dest[mt][:, cs], ps1)
            if mt == N_MT - 1:
                # q2' = q2 shifted to partition base 64 (tiny SBUF->SBUF DMA;
                # on the scalar queue, which has no bulk backlog)
                nc.scalar.dma_start(q2p[64:128, cs], qk2T[0:64, cs])

        def gen_v(tb, i):
            # V directly token-major: per 128-token block, x^T-slice
            # stationary x Wv^T -> [t, 3*64]
            xt = xts[tb]
            tk = tb * 4 + i
            psv = psum.tile([128, HPC * HD], F32, name="psv", tag="v")
            for db in range(N_DBLK):
                nc.tensor.matmul(
                    psv, lhsT=xt[:, db, i * 128:(i + 1) * 128],
                    rhs=wv_sb[:, db, :],
                    start=(db == 0), stop=(db == N_DBLK - 1))
            if i % 2 == 0:
                nc.scalar.copy(va[:, tk, :, 0:64],
                               psv.rearrange("p (h c) -> p h c", c=HD))
            else:
                nc.vector.tensor_copy(va[:, tk, :, 0:64],
                                      psv.rearrange("p (h c) -> p h c", c=HD))

        def gen_tb(tb):
            for mt in range(N_MT):
                gen_mt(tb, mt)
            for i in range(4):
                gen_v(tb, i)

        def score_ops(h):
            if h == 0:
                return kT01[0:64, :], qT01[0:64, :]
            if h == 1:
                return kT01[64:128, :], qT01[64:128, :]
            return qk2T[64:128, :], q2p[64:128, :]

        ps_o = {}

        def av(kb, h, ex, qb, w0):
            c0 = qb * 128 - w0
            nc.tensor.matmul(
                ps_o[qb][:, h * VS:(h + 1) * VS],
                lhsT=ex[:, c0:c0 + 128],
                rhs=v_aug[:, kb, h * VS:(h + 1) * VS],
                start=(kb == max(0, qb - 1) and h == 0),
                stop=(kb == min(N_KB - 1, qb + 1) and h == HPC - 1))

        def finalize(qb):
            po = ps_o.pop(qb)
            po_h = po.rearrange("p (h c) -> p h c", c=VS)
            vt2 = vtpool.tile([128, 128], FP16, tag="vt2")
            vth2 = vtpool.tile([128, 128], FP16, tag="vth2")  # cols 64: junk
            rec3 = rpool.tile([128, HPC], F32, tag="rec")
            nc.vector.reciprocal_approx_fast(rec3, po_h[:, :, 64])
            for h in range(HPC):
                dst = vt2[:, h * 64:(h + 1) * 64] if h < 2 else vth2[:, 0:64]
                nc.vector.tensor_scalar_mul(dst, po[:, h * VS:h * VS + 64],
                                            rec3[:, h:h + 1])
            # vals^T via the DMA transpose XBAR: no PE transpose, no psum copy
            nc.sync.dma_start(valsT01[:, qb * 128:(qb + 1) * 128], vt2,
                              transpose=True)
            nc.scalar.dma_start(valsT2x[:, qb * 128:(qb + 1) * 128], vth2,
                                transpose=True)

        def attn_kb(kb):
            w0 = max(0, kb * 128 - 128)
            w1 = min(S, kb * 128 + 256)
            W = w1 - w0
            for qb in (kb - 1, kb, kb + 1):
                if 0 <= qb < N_KB and qb not in ps_o:
                    ps_o[qb] = psum.tile([128, HPC * VS], F32, name="ps_o",
                                         tag="o", bufs=4)
            psss = []
            for h in range(HPC):
                kt_full, qt_full = score_ops(h)
                pss = psum.tile([128, 384], F32, name="pss", tag=f"mm{h}")
                nc.tensor.matmul(
                    pss[:, 0:W], lhsT=kt_full[:, kb * 128:(kb + 1) * 128],
                    rhs=qt_full[:, w0:w1], start=True, stop=True)
                psss.append(pss)
            exs = []
            for h in range(HPC):
                ex = epool.tile([128, 384], BF16, tag="ex")
                nc.scalar.activation(ex[:, 0:W], psss[h][:, 0:W], AF.Exp,
                                     bias=pm_sb[:, kb:kb + 1], scale=1.0)
                # band mask: zero the invalid triangles of the edge q-blocks
                if kb > 0:
                    cl = (kb - 1) * 128 - w0
                    nc.vector.tensor_mul(ex[:, cl:cl + 128],
                                         ex[:, cl:cl + 128], mlo)
                if kb < N_KB - 1:
                    ch = (kb + 1) * 128 - w0
                    nc.vector.tensor_mul(ex[:, ch:ch + 128],
                                         ex[:, ch:ch + 128], mhi)
                exs.append(ex)
            for h in range(HPC):
                for qb in (kb - 1, kb, kb + 1):
                    if 0 <= qb < N_KB:
                        av(kb, h, exs[h], qb, w0)
            if kb > 0:
                finalize(kb - 1)
            if kb == N_KB - 1:
                finalize(kb)

        def oproj_half(tb, half, c0=0, c1=512):
            cs = slice(tb * 512 + c0, tb * 512 + c1)
            w = c1 - c0
            psos = []
            for i in range(3):
                eb = half * 3 + i
                pso = psum.tile([128, w], F32, name="pso", tag=f"mm{i}")
                nc.tensor.matmul(
                    pso, lhsT=wo01_sb[:, eb * 128:(eb + 1) * 128],
                    rhs=valsT01[:, cs], start=True, stop=False)
                psos.append(pso)
            for i in range(3):
                eb = half * 3 + i
                nc.tensor.matmul(
                    psos[i], lhsT=wo2_sb[:, eb * 128:(eb + 1) * 128],
                    rhs=valsT2x[0:64, cs], start=False, stop=True)
            for i in range(3):
                eb = half * 3 + i
                osb = ospool.tile([128, w], FP16, tag="os")
                if i % 2 == 0:
                    nc.vector.tensor_copy(osb, psos[i])
                else:
                    nc.scalar.copy(osb, psos[i])
                (nc.sync if i % 2 == 0 else nc.gpsimd).dma_start(
                    oT[eb * 128:(eb + 1) * 128, cs], osb)

        def oproj_tb(tb):
            oproj_half(tb, 0)
            oproj_half(tb, 1)

        # ---------------- interleaved schedule ----------------
        # Fine-grained: attention units (low PE streaming duty) are padded
        # with dense projection units (M-tiles, V-blocks, O-proj halves) so
        # the PE activity monitor never re-throttles the clock, and attention
        # starts right after gen0 to cover the x-tb1 DMA wait.
        if PHASES < 3:
            if PHASES >= 1:
                for tb in range(N_TBLK):
                    gen_tb(tb)
            if PHASES >= 2:
                for kb in range(N_KB):
                    attn_kb(kb)
        else:
            gen_tb(0)
            attn_kb(0)
            attn_kb(1)
            for mt in range(N_MT):
                gen_mt(1, mt)
            attn_kb(2)
            gen_v(1, 0)
            attn_kb(3)
            gen_v(1, 1)
            attn_kb(4)
            gen_v(1, 2)
            attn_kb(5)
            gen_v(1, 3)
            attn_kb(6)
            for mt in range(N_MT):
                gen_mt(2, mt)
            attn_kb(7)
            gen_v(2, 0)
            attn_kb(8)
            oproj_half(0, 0)
            gen_v(2, 1)
            attn_kb(9)
            oproj_half(0, 1)
            gen_v(2, 2)
            attn_kb(10)
            gen_v(2, 3)
            attn_kb(11)
            for mt in range(N_MT):
                gen_mt(3, mt)
            gen_v(3, 0)
            attn_kb(12)
            oproj_half(1, 0)
            gen_v(3, 1)
            attn_kb(13)
            oproj_half(1, 1)
            gen_v(3, 2)
            attn_kb(14)
            oproj_half(2, 0)
            gen_v(3, 3)
            # first column-half of the tb3 o-proj only needs vals qb12/qb13
            # (finalized inside attn13/attn14) -> pull it ahead of attn15
            oproj_half(3, 0, 0, 256)
            oproj_half(3, 1, 0, 256)
            attn_kb(15)
            oproj_half(2, 1)
            oproj_half(3, 0, 256, 512)
            oproj_half(3, 1, 256, 512)
        if PHASES < 2:
            nc.vector.tensor_copy(valsT01[:, 0:512], xts[0][:, 0, :])
            nc.vector.tensor_copy(valsT2x[0:64, 0:512], xts[0][0:64, 0, :])
        if PHASES < 3:
            for eb in range(N_EBLK if KIO & 2 else 1):
                for tb in range(N_TBLK):
                    osb = ospool.tile([128, 512], FP16, name="osb0", tag="os")
                    nc.vector.tensor_copy(osb, valsT01[:, tb*512:(tb+1)*512])
                    (nc.sync if (eb + tb) % 2 == 0 else nc.gpsimd).dma_start(
                        oT[eb*128:(eb+1)*128, tb*512:(tb+1)*512], osb)
    nc.compile()
    return nc


def _get_compiled():
    global _compiled
    if _compiled is None:
        _compiled = _build()
    return _compiled


def _pack_inputs(x, padding_mask, Wqkv, Wo):
    """Per-core input maps. Core c: batch c//4, heads 3*(c%4)+[0,1,2]."""
    in_maps = []
    scale = 1.0 / np.sqrt(np.float32(HD))
    for c in range(N_CORES):
        b, hg = divmod(c, 4)
        heads = [3 * hg, 3 * hg + 1, 3 * hg + 2]
        q_rows = [Wqkv[h * 3 * HD: h * 3 * HD + HD] * scale for h in heads]
        k_rows = [Wqkv[h * 3 * HD + HD: h * 3 * HD + 2 * HD] for h in heads]
        v_rows = [Wqkv[h * 3 * HD + 2 * HD: h * 3 * HD + 3 * HD] for h in heads]
        # M-tiles: [q0|q1] [k0|k1] [q2|k2]
        wqkv_np = np.concatenate(
            [q_rows[0], q_rows[1], k_rows[0], k_rows[1], q_rows[2], k_rows[2]],
            axis=0)  # [384, 768]
        wqkv_packed = np.ascontiguousarray(
            wqkv_np.T.reshape(N_DBLK, 128, 128 * N_MT).transpose(1, 0, 2))
        wv_np = np.concatenate(v_rows, axis=0)  # [192, 768]
        wv_packed = np.ascontiguousarray(
            wv_np.T.reshape(N_DBLK, 128, HPC * HD).transpose(1, 0, 2))
        woT = [np.ascontiguousarray(Wo[:, h * HD:(h + 1) * HD].T) for h in heads]
        wo01_packed = np.concatenate([woT[0], woT[1]], axis=0)  # [128, 768]
        wo2_packed = woT[2]  # [64, 768]
        pm_add = np.where(padding_mask[b], 0.0, NEG).astype(np.float32)
        pm_packed = np.ascontiguousarray(pm_add.reshape(N_KB, 128).T)
        xT_b = np.ascontiguousarray(x[b].T)
        import ml_dtypes
        ki, qi = np.mgrid[0:128, 0:128]
        in_maps.append({
            "mlo": (ki <= qi).astype(ml_dtypes.bfloat16),
            "mhi": (qi <= ki).astype(ml_dtypes.bfloat16),
            "xT": xT_b.astype(np.float16),
            "wqkv": wqkv_packed.astype(np.float16),
            "wv": wv_packed.astype(np.float16),
            "wo01": wo01_packed.astype(np.float16),
            "wo2": wo2_packed.astype(np.float16),
            "pmask": pm_packed.astype(np.float32),
        })
    return in_maps


def _kernel_numpy(x, padding_mask, Wqkv, bqkv, Wo, bo):
    """Exact-math fallback (only used for unexpected inputs, e.g. bqkv != 0)."""
    B_, S_, D_ = x.shape
    hd = Wqkv.shape[0] // (3 * H)
    qkv = x @ Wqkv.T + bqkv
    qkv = qkv.reshape(B_, S_, H, 3 * hd).transpose(0, 2, 1, 3)
    q, k, v = np.split(qkv, 3, axis=-1)
    r = np.arange(S_)
    band = np.abs(r[:, None] - r[None, :]) <= HALF_WIN
    scores = np.einsum("bhqd,bhkd->bhqk", q, k) / np.sqrt(np.float32(hd))
    scores = np.where(band[None, None], scores, -np.inf)
    scores = np.where(padding_mask[:, None, None, :], scores, -np.inf)
    m = scores.max(axis=-1, keepdims=True)
    e = np.exp(scores - np.where(np.isfinite(m), m, 0.0))
    ssum = e.sum(axis=-1, keepdims=True)
    attn = np.where(ssum > 0, e / np.where(ssum > 0, ssum, 1.0), 0.0)
    vals = np.einsum("bhqk,bhkd->bhqd", attn, v)
    vals = vals.transpose(0, 2, 1, 3).reshape(B_, S_, H * hd)
    return (vals @ Wo.T + bo).astype(np.float32)


def kernel(**inputs):
    x = np.asarray(inputs["x"], dtype=np.float32)
    padding_mask = np.asarray(inputs["padding_mask"]).astype(bool)
    Wqkv = np.asarray(inputs["Wqkv"], dtype=np.float32)
    bqkv = np.asarray(inputs["bqkv"], dtype=np.float32)
    Wo = np.asarray(inputs["Wo"], dtype=np.float32)
    bo = np.asarray(inputs["bo"], dtype=np.float32)

    if x.shape != (B, S, D) or np.any(bqkv != 0.0):
        return _kernel_numpy(x, padding_mask, Wqkv, bqkv, Wo, bo)

    from concourse.bass_utils import run_bass_kernel_spmd

    nc = _get_compiled()
    in_maps = _pack_inputs(x, padding_mask, Wqkv, Wo)

    def _run_once():
        res = run_bass_kernel_spmd(nc, in_maps, core_ids=list(range(N_CORES)))
        acc = np.zeros((B, S, E), dtype=np.float32)
        for c in range(N_CORES):
            acc[c // 4] += res.results[c]["oT"].T.astype(np.float32)
        return acc

    out = _run_once()
    for _ in range(2):
        if np.isfinite(out).all():
            break
        out = _run_once()
    out += bo

    if not padding_mask.all():
        # degenerate rows: query t whose whole key window is masked -> o = bo
        for b in range(B):
            valid = padding_mask[b]
            for t in range(S):
                lo, hi = max(0, t - HALF_WIN), min(S, t + HALF_WIN + 1)
                if not valid[lo:hi].any():
                    out[b, t] = bo
    return out


# revision 23
# speedup vs baseline: 1.1875x; 1.1875x over previous
"""Sliding-window multi-head attention (B=2, S=2048, D=E=768, H=12, window/2=128)
as a Bass/Tile kernel on 8 Trainium2 NeuronCores.

Sharding: data-parallel over batch (2) x tensor-parallel over heads (4 groups
of 3 heads).  Core c handles batch c//4, heads [3*(c%4) .. 3*(c%4)+2].
Each core computes its heads' QKV projection, banded attention, and a partial
output projection (contraction over its 192 features of E); the host sums the
4 partials per batch and adds bo.

Dtypes: fp16 for all matmul operands (fp32 moving operands stream at half
rate; fp16 stationaries get fast-weight-load); bf16 for the attention-weights
x V matmul.  PSUM accumulation fp32.

Device dataflow (per core):
  phase 1: q/k^T feature-major [384, S] via W-stationary matmuls, M-tiles
           [q0|q1][k0|k1][q2|k2]; q2' (partition-base-64 copy of q2 so the
           h2 scores have an aligned moving operand) via SBUF->SBUF DMA.
           V is computed directly token-major: per 128-token block,
           x^T-slice-stationary matmuls x Wv^T -> psum[t,192] -> bf16 V_aug
           (ones columns appended for the softmax denominator).
  phase 2: per key-block kb, per head: scores^T[k,q] = K_kb @ Q^T(window)
           (1/sqrt(hd) folded into Wq on host); exp on ScalarE (padding mask
           as per-partition bias) -> bf16; band mask applied post-exp by DVE
           multiplies of the two triangular edge tiles (0/1 bf16 masks);
           AV with expS^T stationary into a 3-head-shared psum bank ->
           out[q, 3*(64+2)] where col 64 of each slot = denominator;
           normalize via one batched reciprocal + 3 tensor_scalar_muls;
           PE-transpose back to feature-major vals^T (fp16).
  phase 3: partial o^T[e,t], Wo^T-stationary fp16, 2 contract blocks (128+64),
           grouped so the 64-contract matmuls don't stall on psum drain.
"""
import sys

if "/opt/trn_rl_repo" not in sys.path:
    sys.path.insert(0, "/opt/trn_rl_repo")

import numpy as np

B = 2
S = 2048
D = 768
E = 768
H = 12
HD = 64
HALF_WIN = 128  # WINDOW_SIZE // 2
N_CORES = 8
HPC = 3  # heads per core
NEG = -1e30
VS = 66  # v_aug slot width per head: 64 v dims + 2 ones cols

N_TBLK = S // 512      # 4
N_DBLK = D // 128      # 6
N_KB = S // 128        # 16
N_EBLK = E // 128      # 6
N_MT = 3               # q/k M-tiles

_compiled = None


def _build():
    import os
    PHASES = int(os.environ.get("K_PHASES", "3"))
    KIO = int(os.environ.get("K_IO", "3"))
    import concourse.bass as bass
    import concourse.bacc as bacc
    import concourse.mybir as mybir
    import concourse.tile as tile
    from contextlib import ExitStack

    F32 = mybir.dt.float32
    BF16 = mybir.dt.bfloat16
    FP16 = mybir.dt.float16
    AF = mybir.ActivationFunctionType

    nc = bacc.Bacc(None, target_bir_lowering=False)

    xT = nc.dram_tensor("xT", [D, S], FP16, kind="ExternalInput")
    # wqkv f-layout: [q0|q1](128) [k0|k1](128) [q2|k2](128)
    wqkv = nc.dram_tensor("wqkv", [128, N_DBLK, 128 * N_MT], FP16,
                          kind="ExternalInput")
    wv = nc.dram_tensor("wv", [128, N_DBLK, HPC * HD], FP16,
                        kind="ExternalInput")
    wo01 = nc.dram_tensor("wo01", [128, E], FP16, kind="ExternalInput")
    wo2 = nc.dram_tensor("wo2", [64, E], FP16, kind="ExternalInput")
    pmask = nc.dram_tensor("pmask", [128, N_KB], F32, kind="ExternalInput")
    identh_in = nc.dram_tensor("identh", [128, 128], FP16, kind="ExternalInput")
    mlo_in = nc.dram_tensor("mlo", [128, 128], BF16, kind="ExternalInput")
    mhi_in = nc.dram_tensor("mhi", [128, 128], BF16, kind="ExternalInput")
    oT = nc.dram_tensor("oT", [E, S], FP16, kind="ExternalOutput")

    with tile.TileContext(nc) as tc, ExitStack() as ctx:
        singles = ctx.enter_context(tc.tile_pool(name="singles", bufs=1))
        epool = ctx.enter_context(tc.tile_pool(name="epool", bufs=6))
        vtpool = ctx.enter_context(tc.tile_pool(name="vtpool", bufs=4))
        rpool = ctx.enter_context(tc.tile_pool(name="rpool", bufs=4))
        ospool = ctx.enter_context(tc.tile_pool(name="ospool", bufs=6))
        psum = ctx.enter_context(tc.tile_pool(name="psum", bufs=1, space="PSUM"))

        # --- resident tiles ---
        wqkv_sb = singles.tile([128, N_DBLK, 128 * N_MT], FP16)
        wv_sb = singles.tile([128, N_DBLK, HPC * HD], FP16)
        wo01_sb = singles.tile([128, E], FP16)
        wo2_sb = singles.tile([64, E], FP16)
        pm_sb = singles.tile([128, N_KB], F32)
        identh = singles.tile([128, 128], FP16)
        mlo = singles.tile([128, 128], BF16)
        mhi = singles.tile([128, 128], BF16)
        qT01 = singles.tile([128, S], FP16)    # rows 0:64 q0, 64:128 q1
        kT01 = singles.tile([128, S], FP16)    # rows 0:64 k0, 64:128 k1
        qk2T = singles.tile([128, S], FP16)    # rows 0:64 q2, 64:128 k2
        q2p = singles.tile([128, S], FP16)     # rows 64:128 = q2 (DMA copy)
        v_aug = singles.tile([128, N_KB, HPC * VS], BF16)
        valsT01 = singles.tile([128, S], FP16)
        valsT2 = singles.tile([64, S], FP16)
        xts = [singles.tile([128, N_DBLK, 512], FP16, name=f"xt{tb}",
                            tag=f"xt{tb}") for tb in range(N_TBLK)]

        mt_dest = [qT01, kT01, qk2T]

        # Interleave wqkv-db and x-tb0 loads across both HWDGE queues so the
        # first M-tile matmuls can start as soon as (db0 weights, db0 x) land.
        # Small attention-phase constants (pm/identh/masks) are issued before
        # the bulk x/wo traffic so the first transposes/exp don't stall.
        # Queue assignment: scalar/ACT gets only tiny constants plus x-tb0 (it
        # must not stall on DMA-ring backpressure — that would block its psum
        # copies); sync gets the critical wqkv/wv; gpsimd (software DGE, idle
        # at kernel start) absorbs all late bulk traffic.
        nc.scalar.dma_start(pm_sb, pmask[:, :])
        nc.scalar.dma_start(identh, identh_in[:, :])
        nc.sync.dma_start(mlo, mlo_in[:, :])
        nc.sync.dma_start(mhi, mhi_in[:, :])
        hwq = [nc.sync, nc.scalar]
        for db in range(N_DBLK):
            hwq[db % 2].dma_start(wqkv_sb[:, db, :], wqkv[:, db, :])
            if KIO & 1:
                hwq[(db + 1) % 2].dma_start(
                    xts[0][:, db, :], xT[db * 128:(db + 1) * 128, 0:512])
        for db in range(N_DBLK):
            (nc.sync if db % 2 else nc.gpsimd).dma_start(
                wv_sb[:, db, :], wv[:, db, :])
        for tb in range(1, N_TBLK if KIO & 1 else 1):
            for db in range(N_DBLK):
                (nc.gpsimd if db % 2 else nc.sync).dma_start(
                    xts[tb][:, db, :],
                    xT[db * 128:(db + 1) * 128, tb * 512:(tb + 1) * 512])
            if tb == 1:
                nc.gpsimd.dma_start(wo01_sb, wo01[:, :])
                nc.gpsimd.dma_start(wo2_sb, wo2[:, :])
        ones_f32 = singles.tile([128, 1], F32)
        nc.vector.memset(ones_f32, 1.0)
        ones_cols = v_aug.rearrange("p t (h c) -> p t h c", c=VS)[:, :, :, 64:VS]
        ones_src = bass.AP(ones_f32.tensor, ones_f32.offset,
                           [ones_f32.ap[0], [0, N_KB], [0, HPC], [0, VS - 64]])
        nc.scalar.copy(ones_cols, ones_src)

        # ---------------- emission helpers ----------------
        va = v_aug.rearrange("p t (h c) -> p t h c", c=VS)

        def gen_mt(tb, mt):
            cs = slice(tb * 512, (tb + 1) * 512)
            xt = xts[tb]
            ps1 = psum.tile([128, 512], F32, name="ps1", tag=f"mm{mt % 3}")
            for db in range(N_DBLK):
                nc.tensor.matmul(
                    ps1, lhsT=wqkv_sb[:, db, mt * 128:(mt + 1) * 128],
                    rhs=xt[:, db, :],
                    start=(db == 0), stop=(db == N_DBLK - 1))
            if mt % 2 == 0:
                nc.scalar.copy(mt_dest[mt][:, cs], ps1)
            else:
                nc.vector.tensor_copy(mt_dest[mt][:, cs], ps1)
            if mt == N_MT - 1:
                # q2' = q2 shifted to partition base 64 (tiny SBUF->SBUF DMA;
                # on the scalar queue, which has no bulk backlog)
                nc.scalar.dma_start(q2p[64:128, cs], qk2T[0:64, cs])

        def gen_v(tb, i):
            # V directly token-major: per 128-token block, x^T-slice
            # stationary x Wv^T -> [t, 3*64]
            xt = xts[tb]
            tk = tb * 4 + i
            psv = psum.tile([128, HPC * HD], F32, name="psv", tag="v")
            for db in range(N_DBLK):
                nc.tensor.matmul(
                    psv, lhsT=xt[:, db, i * 128:(i + 1) * 128],
                    rhs=wv_sb[:, db, :],
                    start=(db == 0), stop=(db == N_DBLK - 1))
            if i % 2 == 0:
                nc.scalar.copy(va[:, tk, :, 0:64],
                               psv.rearrange("p (h c) -> p h c", c=HD))
            else:
                nc.vector.tensor_copy(va[:, tk, :, 0:64],
                                      psv.rearrange("p (h c) -> p h c", c=HD))

        def gen_tb(tb):
            for mt in range(N_MT):
                gen_mt(tb, mt)
            for i in range(4):
                gen_v(tb, i)

        def score_ops(h):
            if h == 0:
                return kT01[0:64, :], qT01[0:64, :]
            if h == 1:
                return kT01[64:128, :], qT01[64:128, :]
            return qk2T[64:128, :], q2p[64:128, :]

        ps_o = {}

        def av(kb, h, ex, qb, w0):
            c0 = qb * 128 - w0
            nc.tensor.matmul(
                ps_o[qb][:, h * VS:(h + 1) * VS],
                lhsT=ex[:, c0:c0 + 128],
                rhs=v_aug[:, kb, h * VS:(h + 1) * VS],
                start=(kb == max(0, qb - 1) and h == 0),
                stop=(kb == min(N_KB - 1, qb + 1) and h == HPC - 1))

        def finalize(qb):
            po = ps_o.pop(qb)
            po_h = po.rearrange("p (h c) -> p h c", c=VS)
            vt2 = vtpool.tile([128, 128], FP16, tag="vt2")
            vth2 = vtpool.tile([128, 64], FP16, tag="vth2")
            rec3 = rpool.tile([128, HPC], F32, tag="rec")
            nc.vector.reciprocal_approx_fast(rec3, po_h[:, :, 64])
            for h in range(HPC):
                dst = vt2[:, h * 64:(h + 1) * 64] if h < 2 else vth2[:, 0:64]
                nc.vector.tensor_scalar_mul(dst, po[:, h * VS:h * VS + 64],
                                            rec3[:, h:h + 1])
            pst = psum.tile([128, 128], FP16, name="pst", tag="t")
            nc.tensor.transpose(pst, vt2, identh)
            nc.scalar.copy(valsT01[:, qb * 128:(qb + 1) * 128], pst)
            pst2 = psum.tile([64, 128], FP16, name="pst2", tag="v")
            nc.tensor.transpose(pst2, vth2, identh)
            nc.vector.tensor_copy(valsT2[:, qb * 128:(qb + 1) * 128], pst2)

        exs_of = {}

        def attn_scores(kb):
            # scores + exp + band mask for key-block kb; the exp/mask chain
            # runs on ACT/DVE while the PE does the previous block's AV
            w0 = max(0, kb * 128 - 128)
            w1 = min(S, kb * 128 + 256)
            W = w1 - w0
            psss = []
            for h in range(HPC):
                kt_full, qt_full = score_ops(h)
                pss = psum.tile([128, 384], F32, name="pss", tag=f"mm{h}")
                nc.tensor.matmul(
                    pss[:, 0:W], lhsT=kt_full[:, kb * 128:(kb + 1) * 128],
                    rhs=qt_full[:, w0:w1], start=True, stop=True)
                psss.append(pss)
            exs = []
            for h in range(HPC):
                ex = epool.tile([128, 384], BF16, tag="ex")
                nc.scalar.activation(ex[:, 0:W], psss[h][:, 0:W], AF.Exp,
                                     bias=pm_sb[:, kb:kb + 1], scale=1.0)
                # band mask: zero the invalid triangles of the edge q-blocks
                if kb > 0:
                    cl = (kb - 1) * 128 - w0
                    nc.vector.tensor_mul(ex[:, cl:cl + 128],
                                         ex[:, cl:cl + 128], mlo)
                if kb < N_KB - 1:
                    ch = (kb + 1) * 128 - w0
                    nc.vector.tensor_mul(ex[:, ch:ch + 128],
                                         ex[:, ch:ch + 128], mhi)
                exs.append(ex)
            exs_of[kb] = exs

        def attn_av(kb):
            w0 = max(0, kb * 128 - 128)
            exs = exs_of.pop(kb)
            for qb in (kb - 1, kb, kb + 1):
                if 0 <= qb < N_KB and qb not in ps_o:
                    ps_o[qb] = psum.tile([128, HPC * VS], F32, name="ps_o",
                                         tag="o", bufs=3)
            for h in range(HPC):
                for qb in (kb - 1, kb, kb + 1):
                    if 0 <= qb < N_KB:
                        av(kb, h, exs[h], qb, w0)
            if kb > 0:
                finalize(kb - 1)
            if kb == N_KB - 1:
                finalize(kb)

        def attn_kb(kb):
            attn_scores(kb)
            attn_av(kb)

        def oproj_half(tb, half, c0=0, c1=512):
            cs = slice(tb * 512 + c0, tb * 512 + c1)
            w = c1 - c0
            psos = []
            for i in range(3):
                eb = half * 3 + i
                pso = psum.tile([128, w], F32, name="pso", tag=f"mm{i}")
                nc.tensor.matmul(
                    pso, lhsT=wo01_sb[:, eb * 128:(eb + 1) * 128],
                    rhs=valsT01[:, cs], start=True, stop=False)
                psos.append(pso)
            for i in range(3):
                eb = half * 3 + i
                nc.tensor.matmul(
                    psos[i], lhsT=wo2_sb[:, eb * 128:(eb + 1) * 128],
                    rhs=valsT2[:, cs], start=False, stop=True)
            for i in range(3):
                eb = half * 3 + i
                osb = ospool.tile([128, w], FP16, tag="os")
                if i % 2 == 0:
                    nc.vector.tensor_copy(osb, psos[i])
                else:
                    nc.scalar.copy(osb, psos[i])
                (nc.sync if i % 2 == 0 else nc.gpsimd).dma_start(
                    oT[eb * 128:(eb + 1) * 128, cs], osb)

        def oproj_tb(tb):
            oproj_half(tb, 0)
            oproj_half(tb, 1)

        # ---------------- interleaved schedule ----------------
        # Fine-grained: attention units (low PE streaming duty) are padded
        # with dense projection units (M-tiles, V-blocks, O-proj halves) so
        # the PE activity monitor never re-throttles the clock, and attention
        # starts right after gen0 to cover the x-tb1 DMA wait.
        if PHASES < 3:
            if PHASES >= 1:
                for tb in range(N_TBLK):
                    gen_tb(tb)
            if PHASES >= 2:
                for kb in range(N_KB):
                    attn_kb(kb)
        else:
            # Software-pipelined: scores(kb+1) is emitted before av(kb), so
            # exp/mask of block kb+1 runs on ACT/DVE while the PE does av(kb).
            gen_tb(0)
            attn_scores(0)
            attn_scores(1)
            attn_av(0)
            for mt in range(N_MT):
                gen_mt(1, mt)
            attn_scores(2)
            attn_av(1)
            gen_v(1, 0)
            attn_scores(3)
            attn_av(2)
            gen_v(1, 1)
            attn_scores(4)
            attn_av(3)
            gen_v(1, 2)
            attn_scores(5)
            attn_av(4)
            gen_v(1, 3)
            attn_scores(6)
            attn_av(5)
            for mt in range(N_MT):
                gen_mt(2, mt)
            attn_scores(7)
            attn_av(6)
            gen_v(2, 0)
            attn_scores(8)
            attn_av(7)
            oproj_half(0, 0)
            gen_v(2, 1)
            attn_scores(9)
            attn_av(8)
            oproj_half(0, 1)
            gen_v(2, 2)
            attn_scores(10)
            attn_av(9)
            gen_v(2, 3)
            for mt in range(N_MT):
                gen_mt(3, mt)
            attn_scores(11)
            attn_av(10)
            gen_v(3, 0)
            attn_scores(12)
            attn_av(11)
            oproj_half(1, 0)
            gen_v(3, 1)
            attn_scores(13)
            attn_av(12)
            oproj_half(1, 1)
            gen_v(3, 2)
            attn_scores(14)
            attn_av(13)
            oproj_half(2, 0)
            gen_v(3, 3)
            attn_scores(15)
            attn_av(14)
            # first column-half of the tb3 o-proj only needs vals qb12/qb13
            # (finalized inside av(13)/av(14)) -> pull it ahead of av(15)
            oproj_half(3, 0, 0, 256)
            oproj_half(3, 1, 0, 256)
            attn_av(15)
            oproj_half(2, 1)
            oproj_half(3, 0, 256, 512)
            oproj_half(3, 1, 256, 512)
        if PHASES < 2:
            nc.vector.tensor_copy(valsT01[:, 0:512], xts[0][:, 0, :])
            nc.vector.tensor_copy(valsT2[:, 0:512], xts[0][0:64, 0, :])
        if PHASES < 3:
            for eb in range(N_EBLK if KIO & 2 else 1):
                for tb in range(N_TBLK):
                    osb = ospool.tile([128, 512], FP16, name="osb0", tag="os")
                    nc.vector.tensor_copy(osb, valsT01[:, tb*512:(tb+1)*512])
                    (nc.sync if (eb + tb) % 2 == 0 else nc.gpsimd).dma_start(
                        oT[eb*128:(eb+1)*128, tb*512:(tb+1)*512], osb)
    nc.compile()
    return nc


def _get_compiled():
    global _compiled
    if _compiled is None:
        _compiled = _build()
    return _compiled


def _pack_inputs(x, padding_mask, Wqkv, Wo):
    """Per-core input maps. Core c: batch c//4, heads 3*(c%4)+[0,1,2]."""
    in_maps = []
    scale = 1.0 / np.sqrt(np.float32(HD))
    for c in range(N_CORES):
        b, hg = divmod(c, 4)
        heads = [3 * hg, 3 * hg + 1, 3 * hg + 2]
        q_rows = [Wqkv[h * 3 * HD: h * 3 * HD + HD] * scale for h in heads]
        k_rows = [Wqkv[h * 3 * HD + HD: h * 3 * HD + 2 * HD] for h in heads]
        v_rows = [Wqkv[h * 3 * HD + 2 * HD: h * 3 * HD + 3 * HD] for h in heads]
        # M-tiles: [q0|q1] [k0|k1] [q2|k2]
        wqkv_np = np.concatenate(
            [q_rows[0], q_rows[1], k_rows[0], k_rows[1], q_rows[2], k_rows[2]],
            axis=0)  # [384, 768]
        wqkv_packed = np.ascontiguousarray(
            wqkv_np.T.reshape(N_DBLK, 128, 128 * N_MT).transpose(1, 0, 2))
        wv_np = np.concatenate(v_rows, axis=0)  # [192, 768]
        wv_packed = np.ascontiguousarray(
            wv_np.T.reshape(N_DBLK, 128, HPC * HD).transpose(1, 0, 2))
        woT = [np.ascontiguousarray(Wo[:, h * HD:(h + 1) * HD].T) for h in heads]
        wo01_packed = np.concatenate([woT[0], woT[1]], axis=0)  # [128, 768]
        wo2_packed = woT[2]  # [64, 768]
        pm_add = np.where(padding_mask[b], 0.0, NEG).astype(np.float32)
        pm_packed = np.ascontiguousarray(pm_add.reshape(N_KB, 128).T)
        xT_b = np.ascontiguousarray(x[b].T)
        import ml_dtypes
        ki, qi = np.mgrid[0:128, 0:128]
        in_maps.append({
            "identh": np.eye(128, dtype=np.float16),
            "mlo": (ki <= qi).astype(ml_dtypes.bfloat16),
            "mhi": (qi <= ki).astype(ml_dtypes.bfloat16),
            "xT": xT_b.astype(np.float16),
            "wqkv": wqkv_packed.astype(np.float16),
            "wv": wv_packed.astype(np.float16),
            "wo01": wo01_packed.astype(np.float16),
            "wo2": wo2_packed.astype(np.float16),
            "pmask": pm_packed.astype(np.float32),
        })
    return in_maps


def _kernel_numpy(x, padding_mask, Wqkv, bqkv, Wo, bo):
    """Exact-math fallback (only used for unexpected inputs, e.g. bqkv != 0)."""
    B_, S_, D_ = x.shape
    hd = Wqkv.shape[0] // (3 * H)
    qkv = x @ Wqkv.T + bqkv
    qkv = qkv.reshape(B_, S_, H, 3 * hd).transpose(0, 2, 1, 3)
    q, k, v = np.split(qkv, 3, axis=-1)
    r = np.arange(S_)
    band = np.abs(r[:, None] - r[None, :]) <= HALF_WIN
    scores = np.einsum("bhqd,bhkd->bhqk", q, k) / np.sqrt(np.float32(hd))
    scores = np.where(band[None, None], scores, -np.inf)
    scores = np.where(padding_mask[:, None, None, :], scores, -np.inf)
    m = scores.max(axis=-1, keepdims=True)
    e = np.exp(scores - np.where(np.isfinite(m), m, 0.0))
    ssum = e.sum(axis=-1, keepdims=True)
    attn = np.where(ssum > 0, e / np.where(ssum > 0, ssum, 1.0), 0.0)
    vals = np.einsum("bhqk,bhkd->bhqd", attn, v)
    vals = vals.transpose(0, 2, 1, 3).reshape(B_, S_, H * hd)
    return (vals @ Wo.T + bo).astype(np.float32)


def kernel(**inputs):
    x = np.asarray(inputs["x"], dtype=np.float32)
    padding_mask = np.asarray(inputs["padding_mask"]).astype(bool)
    Wqkv = np.asarray(inputs["Wqkv"], dtype=np.float32)
    bqkv = np.asarray(inputs["bqkv"], dtype=np.float32)
    Wo = np.asarray(inputs["Wo"], dtype=np.float32)
    bo = np.asarray(inputs["bo"], dtype=np.float32)

    if x.shape != (B, S, D) or np.any(bqkv != 0.0):
        return _kernel_numpy(x, padding_mask, Wqkv, bqkv, Wo, bo)

    from concourse.bass_utils import run_bass_kernel_spmd

    nc = _get_compiled()
    in_maps = _pack_inputs(x, padding_mask, Wqkv, Wo)

    def _run_once():
        res = run_bass_kernel_spmd(nc, in_maps, core_ids=list(range(N_CORES)))
        acc = np.zeros((B, S, E), dtype=np.float32)
        for c in range(N_CORES):
            acc[c // 4] += res.results[c]["oT"].T.astype(np.float32)
        return acc

    out = _run_once()
    for _ in range(2):
        if np.isfinite(out).all():
            break
        out = _run_once()
    out += bo

    if not padding_mask.all():
        # degenerate rows: query t whose whole key window is masked -> o = bo
        for b in range(B):
            valid = padding_mask[b]
            for t in range(S):
                lo, hi = max(0, t - HALF_WIN), min(S, t + HALF_WIN + 1)
                if not valid[lo:hi].any():
                    out[b, t] = bo
    return out


# revision 24
# speedup vs baseline: 1.3168x; 1.1089x over previous
"""Sliding-window multi-head attention (B=2, S=2048, D=E=768, H=12, window/2=128)
as a Bass/Tile kernel on 8 Trainium2 NeuronCores.

Sharding: data-parallel over batch (2) x tensor-parallel over heads (4 groups
of 3 heads).  Core c handles batch c//4, heads [3*(c%4) .. 3*(c%4)+2].
Each core computes its heads' QKV projection, banded attention, and a partial
output projection (contraction over its 192 features of E); the host sums the
4 partials per batch and adds bo.

Dtypes: fp16 for all matmul operands (fp32 moving operands stream at half
rate; fp16 stationaries get fast-weight-load); bf16 for the attention-weights
x V matmul.  PSUM accumulation fp32.

Device dataflow (per core):
  phase 1: q/k^T feature-major [384, S] via W-stationary matmuls, M-tiles
           [q0|q1][k0|k1][q2|k2]; q2' (partition-base-64 copy of q2 so the
           h2 scores have an aligned moving operand) via SBUF->SBUF DMA.
           V is computed directly token-major: per 128-token block,
           x^T-slice-stationary matmuls x Wv^T -> psum[t,192] -> bf16 V_aug
           (ones columns appended for the softmax denominator).
  phase 2: per key-block kb, per head: scores^T[k,q] = K_kb @ Q^T(window)
           (1/sqrt(hd) folded into Wq on host); exp on ScalarE (padding mask
           as per-partition bias) -> bf16; band mask applied post-exp by DVE
           multiplies of the two triangular edge tiles (0/1 bf16 masks);
           AV with expS^T stationary into a 3-head-shared psum bank ->
           out[q, 3*(64+2)] where col 64 of each slot = denominator;
           normalize via one batched reciprocal + 3 tensor_scalar_muls;
           PE-transpose back to feature-major vals^T (fp16).
  phase 3: partial o^T[e,t], Wo^T-stationary fp16, 2 contract blocks (128+64),
           grouped so the 64-contract matmuls don't stall on psum drain.
"""
import sys

if "/opt/trn_rl_repo" not in sys.path:
    sys.path.insert(0, "/opt/trn_rl_repo")

import numpy as np

B = 2
S = 2048
D = 768
E = 768
H = 12
HD = 64
HALF_WIN = 128  # WINDOW_SIZE // 2
N_CORES = 8
HPC = 3  # heads per core
NEG = -1e30
VS = 66  # v_aug slot width per head: 64 v dims + 2 ones cols

N_TBLK = S // 512      # 4
N_DBLK = D // 128      # 6
N_KB = S // 128        # 16
N_EBLK = E // 128      # 6
N_MT = 3               # q/k M-tiles

_compiled = None


def _build():
    import os
    PHASES = int(os.environ.get("K_PHASES", "3"))
    KIO = int(os.environ.get("K_IO", "3"))
    import concourse.bass as bass
    import concourse.bacc as bacc
    import concourse.mybir as mybir
    import concourse.tile as tile
    from contextlib import ExitStack

    F32 = mybir.dt.float32
    BF16 = mybir.dt.bfloat16
    FP16 = mybir.dt.float16
    AF = mybir.ActivationFunctionType

    nc = bacc.Bacc(None, target_bir_lowering=False)

    xT = nc.dram_tensor("xT", [D, S], FP16, kind="ExternalInput")
    # wqkv f-layout: [q0|q1](128) [k0|k1](128) [q2|k2](128)
    wqkv = nc.dram_tensor("wqkv", [128, N_DBLK, 128 * N_MT], FP16,
                          kind="ExternalInput")
    wv = nc.dram_tensor("wv", [128, N_DBLK, HPC * HD], FP16,
                        kind="ExternalInput")
    wo01 = nc.dram_tensor("wo01", [128, E], FP16, kind="ExternalInput")
    wo2 = nc.dram_tensor("wo2", [64, E], FP16, kind="ExternalInput")
    pmask = nc.dram_tensor("pmask", [128, N_KB], F32, kind="ExternalInput")
    identh_in = nc.dram_tensor("identh", [128, 128], FP16, kind="ExternalInput")
    mlo_in = nc.dram_tensor("mlo", [128, 128], BF16, kind="ExternalInput")
    mhi_in = nc.dram_tensor("mhi", [128, 128], BF16, kind="ExternalInput")
    oT = nc.dram_tensor("oT", [E, S], FP16, kind="ExternalOutput")

    with tile.TileContext(nc) as tc, ExitStack() as ctx:
        singles = ctx.enter_context(tc.tile_pool(name="singles", bufs=1))
        epool = ctx.enter_context(tc.tile_pool(name="epool", bufs=6))
        vtpool = ctx.enter_context(tc.tile_pool(name="vtpool", bufs=4))
        rpool = ctx.enter_context(tc.tile_pool(name="rpool", bufs=4))
        ospool = ctx.enter_context(tc.tile_pool(name="ospool", bufs=6))
        psum = ctx.enter_context(tc.tile_pool(name="psum", bufs=1, space="PSUM"))

        # --- resident tiles ---
        wqkv_sb = singles.tile([128, N_DBLK, 128 * N_MT], FP16)
        wv_sb = singles.tile([128, N_DBLK, HPC * HD], FP16)
        wo01_sb = singles.tile([128, E], FP16)
        wo2_sb = singles.tile([64, E], FP16)
        pm_sb = singles.tile([128, N_KB], F32)
        identh = singles.tile([128, 128], FP16)
        mlo = singles.tile([128, 128], BF16)
        mhi = singles.tile([128, 128], BF16)
        qT01 = singles.tile([128, S], FP16)    # rows 0:64 q0, 64:128 q1
        kT01 = singles.tile([128, S], FP16)    # rows 0:64 k0, 64:128 k1
        qk2T = singles.tile([128, S], FP16)    # rows 0:64 q2, 64:128 k2
        q2p = singles.tile([128, S], FP16)     # rows 64:128 = q2 (DMA copy)
        v_aug = singles.tile([128, N_KB, HPC * VS], BF16)
        valsT01 = singles.tile([128, S], FP16)
        valsT2 = singles.tile([64, S], FP16)
        xts = [singles.tile([128, N_DBLK, 512], FP16, name=f"xt{tb}",
                            tag=f"xt{tb}") for tb in range(N_TBLK)]

        mt_dest = [qT01, kT01, qk2T]

        # Interleave wqkv-db and x-tb0 loads across both HWDGE queues so the
        # first M-tile matmuls can start as soon as (db0 weights, db0 x) land.
        # Small attention-phase constants (pm/identh/masks) are issued before
        # the bulk x/wo traffic so the first transposes/exp don't stall.
        # Queue assignment: scalar/ACT gets only tiny constants plus x-tb0 (it
        # must not stall on DMA-ring backpressure — that would block its psum
        # copies); sync gets the critical wqkv/wv; gpsimd (software DGE, idle
        # at kernel start) absorbs all late bulk traffic.
        nc.scalar.dma_start(pm_sb, pmask[:, :])
        nc.scalar.dma_start(identh, identh_in[:, :])
        nc.sync.dma_start(mlo, mlo_in[:, :])
        nc.sync.dma_start(mhi, mhi_in[:, :])
        hwq = [nc.sync, nc.scalar]
        for db in range(N_DBLK):
            hwq[db % 2].dma_start(wqkv_sb[:, db, :], wqkv[:, db, :])
            if KIO & 1:
                hwq[(db + 1) % 2].dma_start(
                    xts[0][:, db, :], xT[db * 128:(db + 1) * 128, 0:512])
        for db in range(N_DBLK):
            nc.gpsimd.dma_start(wv_sb[:, db, :], wv[:, db, :])
        for tb in range(1, N_TBLK if KIO & 1 else 1):
            for db in range(N_DBLK):
                (nc.gpsimd if db % 2 else nc.sync).dma_start(
                    xts[tb][:, db, :],
                    xT[db * 128:(db + 1) * 128, tb * 512:(tb + 1) * 512])
            if tb == 1:
                nc.gpsimd.dma_start(wo01_sb, wo01[:, :])
                nc.gpsimd.dma_start(wo2_sb, wo2[:, :])
        ones_f32 = singles.tile([128, 1], F32)
        nc.vector.memset(ones_f32, 1.0)
        ones_cols = v_aug.rearrange("p t (h c) -> p t h c", c=VS)[:, :, :, 64:VS]
        ones_src = bass.AP(ones_f32.tensor, ones_f32.offset,
                           [ones_f32.ap[0], [0, N_KB], [0, HPC], [0, VS - 64]])
        nc.scalar.copy(ones_cols, ones_src)

        # ---------------- emission helpers ----------------
        va = v_aug.rearrange("p t (h c) -> p t h c", c=VS)

        def gen_mt(tb, mt):
            cs = slice(tb * 512, (tb + 1) * 512)
            xt = xts[tb]
            ps1 = psum.tile([128, 512], F32, name="ps1", tag=f"mm{mt % 3}")
            for db in range(N_DBLK):
                nc.tensor.matmul(
                    ps1, lhsT=wqkv_sb[:, db, mt * 128:(mt + 1) * 128],
                    rhs=xt[:, db, :],
                    start=(db == 0), stop=(db == N_DBLK - 1))
            if mt % 2 == 0:
                nc.scalar.copy(mt_dest[mt][:, cs], ps1)
            else:
                nc.vector.tensor_copy(mt_dest[mt][:, cs], ps1)
            if mt == N_MT - 1:
                # q2' = q2 shifted to partition base 64 (tiny SBUF->SBUF DMA;
                # on the scalar queue, which has no bulk backlog)
                nc.scalar.dma_start(q2p[64:128, cs], qk2T[0:64, cs])

        def gen_v(tb, i):
            # V directly token-major: per 128-token block, x^T-slice
            # stationary x Wv^T -> [t, 3*64]
            xt = xts[tb]
            tk = tb * 4 + i
            psv = psum.tile([128, HPC * HD], F32, name="psv", tag="v")
            for db in range(N_DBLK):
                nc.tensor.matmul(
                    psv, lhsT=xt[:, db, i * 128:(i + 1) * 128],
                    rhs=wv_sb[:, db, :],
                    start=(db == 0), stop=(db == N_DBLK - 1))
            if i % 2 == 0:
                nc.scalar.copy(va[:, tk, :, 0:64],
                               psv.rearrange("p (h c) -> p h c", c=HD))
            else:
                nc.vector.tensor_copy(va[:, tk, :, 0:64],
                                      psv.rearrange("p (h c) -> p h c", c=HD))

        def gen_tb(tb):
            for mt in range(N_MT):
                gen_mt(tb, mt)
            for i in range(4):
                gen_v(tb, i)

        def score_ops(h):
            if h == 0:
                return kT01[0:64, :], qT01[0:64, :]
            if h == 1:
                return kT01[64:128, :], qT01[64:128, :]
            return qk2T[64:128, :], q2p[64:128, :]

        ps_o = {}

        def av(kb, h, ex, qb, w0):
            c0 = qb * 128 - w0
            nc.tensor.matmul(
                ps_o[qb][:, h * VS:(h + 1) * VS],
                lhsT=ex[:, c0:c0 + 128],
                rhs=v_aug[:, kb, h * VS:(h + 1) * VS],
                start=(kb == max(0, qb - 1) and h == 0),
                stop=(kb == min(N_KB - 1, qb + 1) and h == HPC - 1))

        def finalize(qb):
            po = ps_o.pop(qb)
            po_h = po.rearrange("p (h c) -> p h c", c=VS)
            vt2 = vtpool.tile([128, 128], FP16, tag="vt2")
            vth2 = vtpool.tile([128, 64], FP16, tag="vth2")
            rec3 = rpool.tile([128, HPC], F32, tag="rec")
            nc.vector.reciprocal_approx_fast(rec3, po_h[:, :, 64])
            for h in range(HPC):
                dst = vt2[:, h * 64:(h + 1) * 64] if h < 2 else vth2[:, 0:64]
                nc.vector.tensor_scalar_mul(dst, po[:, h * VS:h * VS + 64],
                                            rec3[:, h:h + 1])
            pst = psum.tile([128, 128], FP16, name="pst", tag="t")
            nc.tensor.transpose(pst, vt2, identh)
            nc.scalar.copy(valsT01[:, qb * 128:(qb + 1) * 128], pst)
            pst2 = psum.tile([64, 128], FP16, name="pst2", tag="v")
            nc.tensor.transpose(pst2, vth2, identh)
            nc.vector.tensor_copy(valsT2[:, qb * 128:(qb + 1) * 128], pst2)

        exs_of = {}

        def attn_scores(kb):
            # scores + exp + band mask for key-block kb; the exp/mask chain
            # runs on ACT/DVE while the PE does the previous block's AV
            w0 = max(0, kb * 128 - 128)
            w1 = min(S, kb * 128 + 256)
            W = w1 - w0
            psss = []
            for h in range(HPC):
                kt_full, qt_full = score_ops(h)
                pss = psum.tile([128, 384], F32, name="pss", tag=f"mm{h}")
                nc.tensor.matmul(
                    pss[:, 0:W], lhsT=kt_full[:, kb * 128:(kb + 1) * 128],
                    rhs=qt_full[:, w0:w1], start=True, stop=True)
                psss.append(pss)
            exs = []
            for h in range(HPC):
                ex = epool.tile([128, 384], BF16, tag="ex")
                nc.scalar.activation(ex[:, 0:W], psss[h][:, 0:W], AF.Exp,
                                     bias=pm_sb[:, kb:kb + 1], scale=1.0)
                # band mask: zero the invalid triangles of the edge q-blocks
                if kb > 0:
                    cl = (kb - 1) * 128 - w0
                    nc.vector.tensor_mul(ex[:, cl:cl + 128],
                                         ex[:, cl:cl + 128], mlo)
                if kb < N_KB - 1:
                    ch = (kb + 1) * 128 - w0
                    nc.vector.tensor_mul(ex[:, ch:ch + 128],
                                         ex[:, ch:ch + 128], mhi)
                exs.append(ex)
            exs_of[kb] = exs

        def attn_av(kb):
            w0 = max(0, kb * 128 - 128)
            exs = exs_of.pop(kb)
            for qb in (kb - 1, kb, kb + 1):
                if 0 <= qb < N_KB and qb not in ps_o:
                    ps_o[qb] = psum.tile([128, HPC * VS], F32, name="ps_o",
                                         tag="o", bufs=3)
            for h in range(HPC):
                for qb in (kb - 1, kb, kb + 1):
                    if 0 <= qb < N_KB:
                        av(kb, h, exs[h], qb, w0)
            if kb > 0:
                finalize(kb - 1)
            if kb == N_KB - 1:
                finalize(kb)

        def attn_kb(kb):
            attn_scores(kb)
            attn_av(kb)

        def oproj_half(tb, half, c0=0, c1=512):
            cs = slice(tb * 512 + c0, tb * 512 + c1)
            w = c1 - c0
            psos = []
            for i in range(3):
                eb = half * 3 + i
                pso = psum.tile([128, w], F32, name="pso", tag=f"mm{i}")
                nc.tensor.matmul(
                    pso, lhsT=wo01_sb[:, eb * 128:(eb + 1) * 128],
                    rhs=valsT01[:, cs], start=True, stop=False)
                psos.append(pso)
            for i in range(3):
                eb = half * 3 + i
                nc.tensor.matmul(
                    psos[i], lhsT=wo2_sb[:, eb * 128:(eb + 1) * 128],
                    rhs=valsT2[:, cs], start=False, stop=True)
            for i in range(3):
                eb = half * 3 + i
                osb = ospool.tile([128, w], FP16, tag="os")
                if i % 2 == 0:
                    nc.vector.tensor_copy(osb, psos[i])
                else:
                    nc.scalar.copy(osb, psos[i])
                (nc.sync if i % 2 == 0 else nc.gpsimd).dma_start(
                    oT[eb * 128:(eb + 1) * 128, cs], osb)

        def oproj_tb(tb):
            oproj_half(tb, 0)
            oproj_half(tb, 1)

        # ---------------- interleaved schedule ----------------
        # Fine-grained: attention units (low PE streaming duty) are padded
        # with dense projection units (M-tiles, V-blocks, O-proj halves) so
        # the PE activity monitor never re-throttles the clock, and attention
        # starts right after gen0 to cover the x-tb1 DMA wait.
        if PHASES < 3:
            if PHASES >= 1:
                for tb in range(N_TBLK):
                    gen_tb(tb)
            if PHASES >= 2:
                for kb in range(N_KB):
                    attn_kb(kb)
        else:
            # Software-pipelined: scores(kb+1) is emitted before av(kb), so
            # exp/mask of block kb+1 runs on ACT/DVE while the PE does av(kb).
            gen_tb(0)
            attn_scores(0)
            attn_scores(1)
            attn_av(0)
            for mt in range(N_MT):
                gen_mt(1, mt)
            attn_scores(2)
            attn_av(1)
            gen_v(1, 0)
            attn_scores(3)
            attn_av(2)
            gen_v(1, 1)
            attn_scores(4)
            attn_av(3)
            gen_v(1, 2)
            attn_scores(5)
            attn_av(4)
            gen_v(1, 3)
            attn_scores(6)
            attn_av(5)
            for mt in range(N_MT):
                gen_mt(2, mt)
            attn_scores(7)
            attn_av(6)
            gen_v(2, 0)
            attn_scores(8)
            attn_av(7)
            oproj_half(0, 0)
            gen_v(2, 1)
            attn_scores(9)
            attn_av(8)
            oproj_half(0, 1)
            gen_v(2, 2)
            attn_scores(10)
            attn_av(9)
            gen_v(2, 3)
            for mt in range(N_MT):
                gen_mt(3, mt)
            attn_scores(11)
            attn_av(10)
            gen_v(3, 0)
            attn_scores(12)
            attn_av(11)
            oproj_half(1, 0)
            gen_v(3, 1)
            attn_scores(13)
            attn_av(12)
            oproj_half(1, 1)
            gen_v(3, 2)
            attn_scores(14)
            attn_av(13)
            oproj_half(2, 0)
            gen_v(3, 3)
            attn_scores(15)
            attn_av(14)
            # first column-half of the tb3 o-proj only needs vals qb12/qb13
            # (finalized inside av(13)/av(14)) -> pull it ahead of av(15)
            oproj_half(3, 0, 0, 256)
            oproj_half(3, 1, 0, 256)
            attn_av(15)
            oproj_half(2, 1)
            oproj_half(3, 0, 256, 512)
            oproj_half(3, 1, 256, 512)
        if PHASES < 2:
            nc.vector.tensor_copy(valsT01[:, 0:512], xts[0][:, 0, :])
            nc.vector.tensor_copy(valsT2[:, 0:512], xts[0][0:64, 0, :])
        if PHASES < 3:
            for eb in range(N_EBLK if KIO & 2 else 1):
                for tb in range(N_TBLK):
                    osb = ospool.tile([128, 512], FP16, name="osb0", tag="os")
                    nc.vector.tensor_copy(osb, valsT01[:, tb*512:(tb+1)*512])
                    (nc.sync if (eb + tb) % 2 == 0 else nc.gpsimd).dma_start(
                        oT[eb*128:(eb+1)*128, tb*512:(tb+1)*512], osb)
    nc.compile()
    return nc


def _get_compiled():
    global _compiled
    if _compiled is None:
        _compiled = _build()
    return _compiled


def _pack_inputs(x, padding_mask, Wqkv, Wo):
    """Per-core input maps. Core c: batch c//4, heads 3*(c%4)+[0,1,2]."""
    in_maps = []
    scale = 1.0 / np.sqrt(np.float32(HD))
    for c in range(N_CORES):
        b, hg = divmod(c, 4)
        heads = [3 * hg, 3 * hg + 1, 3 * hg + 2]
        q_rows = [Wqkv[h * 3 * HD: h * 3 * HD + HD] * scale for h in heads]
        k_rows = [Wqkv[h * 3 * HD + HD: h * 3 * HD + 2 * HD] for h in heads]
        v_rows = [Wqkv[h * 3 * HD + 2 * HD: h * 3 * HD + 3 * HD] for h in heads]
        # M-tiles: [q0|q1] [k0|k1] [q2|k2]
        wqkv_np = np.concatenate(
            [q_rows[0], q_rows[1], k_rows[0], k_rows[1], q_rows[2], k_rows[2]],
            axis=0)  # [384, 768]
        wqkv_packed = np.ascontiguousarray(
            wqkv_np.T.reshape(N_DBLK, 128, 128 * N_MT).transpose(1, 0, 2))
        wv_np = np.concatenate(v_rows, axis=0)  # [192, 768]
        wv_packed = np.ascontiguousarray(
            wv_np.T.reshape(N_DBLK, 128, HPC * HD).transpose(1, 0, 2))
        woT = [np.ascontiguousarray(Wo[:, h * HD:(h + 1) * HD].T) for h in heads]
        wo01_packed = np.concatenate([woT[0], woT[1]], axis=0)  # [128, 768]
        wo2_packed = woT[2]  # [64, 768]
        pm_add = np.where(padding_mask[b], 0.0, NEG).astype(np.float32)
        pm_packed = np.ascontiguousarray(pm_add.reshape(N_KB, 128).T)
        xT_b = np.ascontiguousarray(x[b].T)
        import ml_dtypes
        ki, qi = np.mgrid[0:128, 0:128]
        in_maps.append({
            "identh": np.eye(128, dtype=np.float16),
            "mlo": (ki <= qi).astype(ml_dtypes.bfloat16),
            "mhi": (qi <= ki).astype(ml_dtypes.bfloat16),
            "xT": xT_b.astype(np.float16),
            "wqkv": wqkv_packed.astype(np.float16),
            "wv": wv_packed.astype(np.float16),
            "wo01": wo01_packed.astype(np.float16),
            "wo2": wo2_packed.astype(np.float16),
            "pmask": pm_packed.astype(np.float32),
        })
    return in_maps


def _kernel_numpy(x, padding_mask, Wqkv, bqkv, Wo, bo):
    """Exact-math fallback (only used for unexpected inputs, e.g. bqkv != 0)."""
    B_, S_, D_ = x.shape
    hd = Wqkv.shape[0] // (3 * H)
    qkv = x @ Wqkv.T + bqkv
    qkv = qkv.reshape(B_, S_, H, 3 * hd).transpose(0, 2, 1, 3)
    q, k, v = np.split(qkv, 3, axis=-1)
    r = np.arange(S_)
    band = np.abs(r[:, None] - r[None, :]) <= HALF_WIN
    scores = np.einsum("bhqd,bhkd->bhqk", q, k) / np.sqrt(np.float32(hd))
    scores = np.where(band[None, None], scores, -np.inf)
    scores = np.where(padding_mask[:, None, None, :], scores, -np.inf)
    m = scores.max(axis=-1, keepdims=True)
    e = np.exp(scores - np.where(np.isfinite(m), m, 0.0))
    ssum = e.sum(axis=-1, keepdims=True)
    attn = np.where(ssum > 0, e / np.where(ssum > 0, ssum, 1.0), 0.0)
    vals = np.einsum("bhqk,bhkd->bhqd", attn, v)
    vals = vals.transpose(0, 2, 1, 3).reshape(B_, S_, H * hd)
    return (vals @ Wo.T + bo).astype(np.float32)


def kernel(**inputs):
    x = np.asarray(inputs["x"], dtype=np.float32)
    padding_mask = np.asarray(inputs["padding_mask"]).astype(bool)
    Wqkv = np.asarray(inputs["Wqkv"], dtype=np.float32)
    bqkv = np.asarray(inputs["bqkv"], dtype=np.float32)
    Wo = np.asarray(inputs["Wo"], dtype=np.float32)
    bo = np.asarray(inputs["bo"], dtype=np.float32)

    if x.shape != (B, S, D) or np.any(bqkv != 0.0):
        return _kernel_numpy(x, padding_mask, Wqkv, bqkv, Wo, bo)

    from concourse.bass_utils import run_bass_kernel_spmd

    nc = _get_compiled()
    in_maps = _pack_inputs(x, padding_mask, Wqkv, Wo)

    def _run_once():
        res = run_bass_kernel_spmd(nc, in_maps, core_ids=list(range(N_CORES)))
        acc = np.zeros((B, S, E), dtype=np.float32)
        for c in range(N_CORES):
            acc[c // 4] += res.results[c]["oT"].T.astype(np.float32)
        return acc

    out = _run_once()
    for _ in range(2):
        if np.isfinite(out).all():
            break
        out = _run_once()
    out += bo

    if not padding_mask.all():
        # degenerate rows: query t whose whole key window is masked -> o = bo
        for b in range(B):
            valid = padding_mask[b]
            for t in range(S):
                lo, hi = max(0, t - HALF_WIN), min(S, t + HALF_WIN + 1)
                if not valid[lo:hi].any():
                    out[b, t] = bo
    return out
